# revision 22
# baseline (speedup 1.0000x reference)
"""Trainium2 Bass kernel for differentiable KDE (Gaussian kernel density estimate).

Math (h = 1):
    density[i] = (1/M) * sum_j exp(-C * ||x_i - d_j||^2),  C = 0.5 / sqrt(2*pi)
               = exp(-C||x_i||^2)/M * sum_j exp(2C x_i.d_j - C||d_j||^2)

Sharding: 4 x-shards x 2 data-shards over 8 cores. Each core computes
    root[p, i] = sum_t exp(2C x_i . d_{j0+128t+p} - C||d_{j0+128t+p}||^2)
(bf16 [128, 2048]); the host sums the two j-shards' roots over partitions and
applies exp(-C||x_i||^2)/M.

Per-core pipeline (all transposes/norms precomputed on host):
    - PE: one wide fp16 matmul per j-tile: psum[j=128, i=2048] = dT_t.T @ xT.
    - exp is split across two engines: ACT computes exact exp (scale=2C,
      per-partition bias -C||d_j||^2) for most tiles; for NSCHR of the 32
      tiles the DVE computes a Schraudolph-style exp approximation in one
      tensor_scalar pass: round(arg*log2(e)*128 + (16256-c)) written as int16
      IS the bf16 bit pattern of ~exp(arg). (Error ~0.8% on those tiles'
      terms; tiles are interleaved so the blended error stays ~0.1%.)
    - DVE merges all 32 e-tiles with scalar_tensor_tensor adds (4x mode) into
      4 interleaved bf16 accumulators, merged at the end and DMA'd out.
"""
import math

import numpy as np

from concourse import bacc, mybir, tile
from concourse.bass_utils import run_bass_kernel_spmd

N, M, D = 8192, 8192, 128
P = 128
GI, GJ = 4, 2               # core grid: 4 x-shards x 2 data-shards
NCORES = GI * GJ
NI = N // GI                # 2048 x-rows per core
MJ = M // GJ                # 4096 data rows per core
NTJ = MJ // P               # 32 j-tiles
CHUNK_TILES = [1, 1, 2, 2, 2, 2, 4, 4, 4, 4, 2, 2, 2]  # 32 j-tiles total
MMW = 512                   # matmul free width (psum bank limit)

C = 0.5 / math.sqrt(2.0 * math.pi)
TWO_C = 2.0 * C
LOG2E = 1.4426950408889634
SCHR_C = 7.3                # Schraudolph bias constant (tuned in numpy)
SCHR_SCALE = TWO_C * LOG2E * 128.0

# j-tile exp engine assignment: ACT computes exact exp on 24 tiles, DVE the
# Schraudolph approximation on 8; GPSIMD (no PSUM access) instead owns the
# bf16 adds of accumulator 3 (tiles jt%4==3)
DVE_TILES = {3, 6, 9, 12, 15, 18, 21, 24, 27, 30}
GPS_TILES = set()

F32 = mybir.dt.float32
F32R = mybir.dt.float32r
F16 = mybir.dt.float16
I16 = mybir.dt.int16
BF16 = mybir.dt.bfloat16

_CACHED_NC = None


def _patch_ldw_opt():
    from concourse import bass_utils as _bu
    if getattr(_bu, "_ldw_patched", False):
        return
    _orig = _bu.run_command

    def _patched(cmd, *a, **kw):
        if isinstance(cmd, list):
            cmd = [c.replace("--enable-ldw-opt=false", "--enable-ldw-opt=true")
                   if isinstance(c, str) else c for c in cmd]
        return _orig(cmd, *a, **kw)

    _bu.run_command = _patched
    _bu._ldw_patched = True


def _build():
    _patch_ldw_opt()
    nc = bacc.Bacc("TRN2", target_bir_lowering=False, debug=False)
    x_d = nc.dram_tensor("xT", [P, NI], F32R, kind="ExternalInput")
    d_d = nc.dram_tensor("dT", [P, MJ], F32R, kind="ExternalInput")
    b_d = nc.dram_tensor("db", [P, NTJ], F32, kind="ExternalInput")
    s_d = nc.dram_tensor("dbs", [P, NTJ], F32, kind="ExternalInput")
    o_d = nc.dram_tensor("root", [P, (NTJ // 2) * NI], BF16, kind="ExternalOutput")

    with tile.TileContext(nc) as tc:
        with tc.tile_pool(name="xbuf", bufs=4) as xbuf_pool, \
             tc.tile_pool(name="dbuf", bufs=1) as dbuf_pool, \
             tc.tile_pool(name="bias", bufs=1) as bias_pool, \
             tc.tile_pool(name="e", bufs=7) as e_pool, \
             tc.tile_pool(name="lvl", bufs=4) as lvl_pool, \
             tc.tile_pool(name="psm", bufs=2, space="PSUM") as ps_main:

            db = bias_pool.tile([P, NTJ], F32, tag="db")
            dbs = bias_pool.tile([P, NTJ], F32, tag="dbs")
            one_bf = bias_pool.tile([P, 1], BF16, tag="onebf")
            nc.gpsimd.memset(one_bf[:], 1.0)
            dma_engines = [nc.sync, nc.scalar, nc.gpsimd]
            # critical path: first matmul needs xc0 + dT tile 0 only
            xcs = []
            for xc in range(4):
                xt_c = xbuf_pool.tile([P, 512], F32R, tag=f"xc{xc}")
                xcs.append(xt_c)
            chunk_tiles = []   # tile-index -> (chunk tile, offset within chunk)
            chunk_aps = []
            j0 = 0
            for ci, ctiles in enumerate(CHUNK_TILES):
                dchunk = dbuf_pool.tile([P, ctiles * P], F32R, tag=f"dch{ci}")
                chunk_aps.append((dchunk, j0, ctiles))
                for t in range(ctiles):
                    chunk_tiles.append((dchunk, t))
                j0 += ctiles
            # issue order: ch0 -> sync, xc0 -> scalar, ch1 -> gpsimd, then
            # round-robin the rest behind
            nc.scalar.dma_start(chunk_aps[0][0][:],
                                d_d.ap()[:, 0:CHUNK_TILES[0] * P])
            nc.sync.dma_start(xcs[0][:], x_d.ap()[:, 0:512])
            nc.gpsimd.dma_start(chunk_aps[1][0][:],
                                d_d.ap()[:, P:(1 + CHUNK_TILES[1]) * P])
            nc.sync.dma_start(xcs[1][:], x_d.ap()[:, 512:1024])
            nc.gpsimd.dma_start(xcs[2][:], x_d.ap()[:, 1024:1536])
            nc.sync.dma_start(xcs[3][:], x_d.ap()[:, 1536:2048])
            nc.gpsimd.dma_start(db[:], b_d.ap())
            nc.gpsimd.dma_start(dbs[:], s_d.ap())
            eng2 = [nc.sync, nc.gpsimd]
            for ci in range(2, len(CHUNK_TILES)):
                dchunk, cj0, ctiles = chunk_aps[ci]
                eng2[ci % 2].dma_start(
                    dchunk[:], d_d.ap()[:, cj0 * P:(cj0 + ctiles) * P])

            # pair accumulators: acc_g = e_{2g} + e_{2g+1}, DMA'd out as
            # soon as ready; host finishes the reduction
            out_engines = [nc.sync, nc.gpsimd, nc.sync, nc.gpsimd, nc.scalar]
            prev_e = [None]

            for jt in range(NTJ):
                cht, off = chunk_tiles[jt]
                dsl = cht[:, off * P:(off + 1) * P]
                pm = ps_main.tile([P, NI], F32, tag="pm")
                for wc in range(NI // MMW):
                    sl = slice(wc * MMW, (wc + 1) * MMW)
                    nc.tensor.matmul(pm[:, sl], dsl, xcs[wc][:],
                                     start=True, stop=True)
                if jt in DVE_TILES or jt in GPS_TILES:
                    eng = nc.vector if jt in DVE_TILES else nc.gpsimd
                    eb = e_pool.tile([P, NI], BF16, tag="e")
                    eng.tensor_scalar(
                        eb[:].bitcast(I16), pm[:], SCHR_SCALE, dbs[:, jt:jt + 1],
                        op0=mybir.AluOpType.mult, op1=mybir.AluOpType.add)
                    e = eb[:]
                else:
                    eb = e_pool.tile([P, NI], BF16, tag="e")
                    nc.scalar.activation(eb[:], pm[:],
                                         mybir.ActivationFunctionType.Exp,
                                         bias=db[:, jt:jt + 1], scale=TWO_C)
                    e = eb[:]
                if jt % 2 == 0:
                    prev_e[0] = e
                else:
                    g = jt // 2
                    pacc = lvl_pool.tile([P, NI], BF16, tag="pacc")
                    nc.vector.tensor_add(pacc[:], prev_e[0], e)
                    out_engines[g % 5].dma_start(
                        o_d.ap()[:, g * NI:(g + 1) * NI], pacc[:])



    nc.compile()
    return nc


def make_in_maps(x, data):
    """Host prep: transpose/shard/cast inputs. Returns (in_maps, xfac[N])."""
    x = np.ascontiguousarray(np.asarray(x, dtype=np.float32))
    data = np.ascontiguousarray(np.asarray(data, dtype=np.float32))
    assert x.shape == (N, D) and data.shape == (M, D)

    xT = np.ascontiguousarray(x.T)                      # [128, N] f32
    dT = np.ascontiguousarray(data.T)                   # [128, M] f32
    dnsq = np.einsum("md,md->m", data.astype(np.float64),
                     data.astype(np.float64))           # [M]
    db_full = (-C * dnsq).reshape(M // P, P).T.astype(np.float32)
    dbs_full = (db_full.astype(np.float64) * (LOG2E * 128.0)
                + (127.0 * 128.0 - SCHR_C)).astype(np.float32)

    xnsq = np.einsum("nd,nd->n", x.astype(np.float64), x.astype(np.float64))
    xfac = np.exp(-C * xnsq) / float(M)                 # [N] f64

    in_maps = []
    for c in range(NCORES):
        gi, gj = c // GJ, c % GJ
        in_maps.append({
            "xT": np.ascontiguousarray(xT[:, gi * NI:(gi + 1) * NI]),
            "dT": np.ascontiguousarray(dT[:, gj * MJ:(gj + 1) * MJ]),
            "db": np.ascontiguousarray(db_full[:, gj * NTJ:(gj + 1) * NTJ]),
            "dbs": np.ascontiguousarray(dbs_full[:, gj * NTJ:(gj + 1) * NTJ]),
        })
    return in_maps, xfac


def kernel(x, data):
    global _CACHED_NC
    if _CACHED_NC is None:
        _CACHED_NC = _build()
    nc = _CACHED_NC

    in_maps, xfac = make_in_maps(x, data)
    res = run_bass_kernel_spmd(nc, in_maps, list(range(NCORES)))

    dens = np.empty(N, dtype=np.float64)
    for gi in range(GI):
        s = np.zeros(NI, dtype=np.float64)
        for gj in range(GJ):
            root = np.asarray(res.results[gi * GJ + gj]["root"])
            s += root.astype(np.float64).reshape(P, NTJ // 2, NI).sum(axis=(0, 1))
        sl = slice(gi * NI, (gi + 1) * NI)
        dens[sl] = s * xfac[sl]
    return dens.reshape(N, 1).astype(np.float32)


if __name__ == "__main__":
    rng = np.random.default_rng(0)
    x = rng.standard_normal((N, D), dtype=np.float32)
    data = rng.standard_normal((M, D), dtype=np.float32)
    out = kernel(x, data)
    print("kernel out", out.shape, out[:4, 0])


# revision 23
# speedup vs baseline: 1.1557x; 1.1557x over previous
"""Trainium2 Bass kernel for differentiable KDE (Gaussian kernel density estimate).

Math (h = 1):
    density[i] = (1/M) * sum_j exp(-C * ||x_i - d_j||^2),  C = 0.5 / sqrt(2*pi)
               = exp(-C||x_i||^2)/M * sum_j exp(2C x_i.d_j - C||d_j||^2)

Sharding: 4 x-shards x 2 data-shards over 8 cores. Each core computes
    root[p, i] = sum_t exp(2C x_i . d_{j0+128t+p} - C||d_{j0+128t+p}||^2)
(bf16 [128, 2048]); the host sums the two j-shards' roots over partitions and
applies exp(-C||x_i||^2)/M.

Per-core pipeline (all transposes/norms precomputed on host):
    - PE: one wide fp16 matmul per j-tile: psum[j=128, i=2048] = dT_t.T @ xT.
    - exp is split across two engines: ACT computes exact exp (scale=2C,
      per-partition bias -C||d_j||^2) for most tiles; for NSCHR of the 32
      tiles the DVE computes a Schraudolph-style exp approximation in one
      tensor_scalar pass: round(arg*log2(e)*128 + (16256-c)) written as int16
      IS the bf16 bit pattern of ~exp(arg). (Error ~0.8% on those tiles'
      terms; tiles are interleaved so the blended error stays ~0.1%.)
    - DVE merges all 32 e-tiles with scalar_tensor_tensor adds (4x mode) into
      4 interleaved bf16 accumulators, merged at the end and DMA'd out.
"""
import math

import numpy as np

from concourse import bacc, mybir, tile
from concourse.bass_utils import run_bass_kernel_spmd

N, M, D = 8192, 8192, 128
P = 128
GI, GJ = 4, 2               # core grid: 4 x-shards x 2 data-shards
NCORES = GI * GJ
NI = N // GI                # 2048 x-rows per core
MJ = M // GJ                # 4096 data rows per core
NTJ = MJ // P               # 32 j-tiles
CHUNK_TILES = [1, 1, 2, 2, 2, 2, 4, 4, 4, 4, 2, 2, 2]  # 32 j-tiles total
MMW = 512                   # matmul free width (psum bank limit)

C = 0.5 / math.sqrt(2.0 * math.pi)
TWO_C = 2.0 * C
LOG2E = 1.4426950408889634
SCHR_C = 7.3                # Schraudolph bias constant (tuned in numpy)
SCHR_SCALE = TWO_C * LOG2E * 128.0

# j-tile exp engine assignment: ACT computes exact exp on 24 tiles, DVE the
# Schraudolph approximation on 8; GPSIMD (no PSUM access) instead owns the
# bf16 adds of accumulator 3 (tiles jt%4==3)
DVE_TILES = {3, 6, 9, 12, 15, 18, 21, 24, 27, 30}
GPS_TILES = set()

F32 = mybir.dt.float32
F32R = mybir.dt.float32r
F16 = mybir.dt.float16
I16 = mybir.dt.int16
BF16 = mybir.dt.bfloat16

_CACHED_NC = None


def _patch_ldw_opt():
    from concourse import bass_utils as _bu
    if getattr(_bu, "_ldw_patched", False):
        return
    _orig = _bu.run_command

    def _patched(cmd, *a, **kw):
        if isinstance(cmd, list):
            cmd = [c.replace("--enable-ldw-opt=false", "--enable-ldw-opt=true")
                   if isinstance(c, str) else c for c in cmd]
        return _orig(cmd, *a, **kw)

    _bu.run_command = _patched
    _bu._ldw_patched = True


def _build():
    _patch_ldw_opt()
    nc = bacc.Bacc("TRN2", target_bir_lowering=False, debug=False)
    x_d = nc.dram_tensor("xT", [P, NI], F32R, kind="ExternalInput")
    d_d = nc.dram_tensor("dT", [P, MJ], F32R, kind="ExternalInput")
    b_d = nc.dram_tensor("db", [P, NTJ], F32, kind="ExternalInput")
    s_d = nc.dram_tensor("dbs", [P, NTJ], F32, kind="ExternalInput")
    o_d = nc.dram_tensor("root", [P, (NTJ // 2) * NI], BF16, kind="ExternalOutput")

    with tile.TileContext(nc) as tc:
        with tc.tile_pool(name="xbuf", bufs=4) as xbuf_pool, \
             tc.tile_pool(name="dbuf", bufs=1) as dbuf_pool, \
             tc.tile_pool(name="bias", bufs=1) as bias_pool, \
             tc.tile_pool(name="e", bufs=7) as e_pool, \
             tc.tile_pool(name="lvl", bufs=4) as lvl_pool, \
             tc.tile_pool(name="psm", bufs=4, space="PSUM") as ps_main:

            db = bias_pool.tile([P, NTJ], F32, tag="db")
            dbs = bias_pool.tile([P, NTJ], F32, tag="dbs")
            one_bf = bias_pool.tile([P, 1], BF16, tag="onebf")
            nc.gpsimd.memset(one_bf[:], 1.0)
            dma_engines = [nc.sync, nc.scalar, nc.gpsimd]
            # critical path: first matmul needs xc0 + dT tile 0 only
            xcs = []
            for xc in range(4):
                xt_c = xbuf_pool.tile([P, 512], F32R, tag=f"xc{xc}")
                xcs.append(xt_c)
            chunk_tiles = []   # tile-index -> (chunk tile, offset within chunk)
            chunk_aps = []
            j0 = 0
            for ci, ctiles in enumerate(CHUNK_TILES):
                dchunk = dbuf_pool.tile([P, ctiles * P], F32R, tag=f"dch{ci}")
                chunk_aps.append((dchunk, j0, ctiles))
                for t in range(ctiles):
                    chunk_tiles.append((dchunk, t))
                j0 += ctiles
            # issue order: ch0 -> sync, xc0 -> scalar, ch1 -> gpsimd, then
            # round-robin the rest behind
            nc.scalar.dma_start(chunk_aps[0][0][:],
                                d_d.ap()[:, 0:CHUNK_TILES[0] * P])
            nc.sync.dma_start(xcs[0][:], x_d.ap()[:, 0:512])
            nc.gpsimd.dma_start(chunk_aps[1][0][:],
                                d_d.ap()[:, P:(1 + CHUNK_TILES[1]) * P])
            nc.sync.dma_start(xcs[1][:], x_d.ap()[:, 512:1024])
            nc.gpsimd.dma_start(xcs[2][:], x_d.ap()[:, 1024:1536])
            nc.sync.dma_start(xcs[3][:], x_d.ap()[:, 1536:2048])
            nc.gpsimd.dma_start(db[:], b_d.ap())
            nc.gpsimd.dma_start(dbs[:], s_d.ap())
            eng2 = [nc.sync, nc.gpsimd]
            for ci in range(2, len(CHUNK_TILES)):
                dchunk, cj0, ctiles = chunk_aps[ci]
                eng2[ci % 2].dma_start(
                    dchunk[:], d_d.ap()[:, cj0 * P:(cj0 + ctiles) * P])

            # pair accumulators: acc_g = e_{2g} + e_{2g+1}, DMA'd out as
            # soon as ready; host finishes the reduction
            out_engines = [nc.sync, nc.gpsimd, nc.sync, nc.gpsimd, nc.scalar]
            prev_e = [None]

            for jt in range(NTJ):
                cht, off = chunk_tiles[jt]
                dsl = cht[:, off * P:(off + 1) * P]
                eb = e_pool.tile([P, NI], BF16, tag="e")
                for h in range(2):
                    pm = ps_main.tile([P, NI // 2], F32, tag="pm")
                    for wc in range(2):
                        sl = slice(wc * MMW, (wc + 1) * MMW)
                        nc.tensor.matmul(pm[:, sl], dsl, xcs[2 * h + wc][:],
                                         start=True, stop=True)
                    esl = eb[:, h * (NI // 2):(h + 1) * (NI // 2)]
                    if jt in DVE_TILES:
                        nc.vector.tensor_scalar(
                            esl.bitcast(I16), pm[:], SCHR_SCALE,
                            dbs[:, jt:jt + 1],
                            op0=mybir.AluOpType.mult, op1=mybir.AluOpType.add)
                    else:
                        nc.scalar.activation(esl, pm[:],
                                             mybir.ActivationFunctionType.Exp,
                                             bias=db[:, jt:jt + 1], scale=TWO_C)
                e = eb[:]
                if jt % 2 == 0:
                    prev_e[0] = e
                else:
                    g = jt // 2
                    pacc = lvl_pool.tile([P, NI], BF16, tag="pacc")
                    nc.vector.tensor_add(pacc[:], prev_e[0], e)
                    out_engines[g % 5].dma_start(
                        o_d.ap()[:, g * NI:(g + 1) * NI], pacc[:])



    nc.compile()
    return nc


def make_in_maps(x, data):
    """Host prep: transpose/shard/cast inputs. Returns (in_maps, xfac[N])."""
    x = np.ascontiguousarray(np.asarray(x, dtype=np.float32))
    data = np.ascontiguousarray(np.asarray(data, dtype=np.float32))
    assert x.shape == (N, D) and data.shape == (M, D)

    xT = np.ascontiguousarray(x.T)                      # [128, N] f32
    dT = np.ascontiguousarray(data.T)                   # [128, M] f32
    dnsq = np.einsum("md,md->m", data.astype(np.float64),
                     data.astype(np.float64))           # [M]
    db_full = (-C * dnsq).reshape(M // P, P).T.astype(np.float32)
    dbs_full = (db_full.astype(np.float64) * (LOG2E * 128.0)
                + (127.0 * 128.0 - SCHR_C)).astype(np.float32)

    xnsq = np.einsum("nd,nd->n", x.astype(np.float64), x.astype(np.float64))
    xfac = np.exp(-C * xnsq) / float(M)                 # [N] f64

    in_maps = []
    for c in range(NCORES):
        gi, gj = c // GJ, c % GJ
        in_maps.append({
            "xT": np.ascontiguousarray(xT[:, gi * NI:(gi + 1) * NI]),
            "dT": np.ascontiguousarray(dT[:, gj * MJ:(gj + 1) * MJ]),
            "db": np.ascontiguousarray(db_full[:, gj * NTJ:(gj + 1) * NTJ]),
            "dbs": np.ascontiguousarray(dbs_full[:, gj * NTJ:(gj + 1) * NTJ]),
        })
    return in_maps, xfac


def kernel(x, data):
    global _CACHED_NC
    if _CACHED_NC is None:
        _CACHED_NC = _build()
    nc = _CACHED_NC

    in_maps, xfac = make_in_maps(x, data)
    res = run_bass_kernel_spmd(nc, in_maps, list(range(NCORES)))

    dens = np.empty(N, dtype=np.float64)
    for gi in range(GI):
        s = np.zeros(NI, dtype=np.float64)
        for gj in range(GJ):
            root = np.asarray(res.results[gi * GJ + gj]["root"])
            s += root.astype(np.float64).reshape(P, NTJ // 2, NI).sum(axis=(0, 1))
        sl = slice(gi * NI, (gi + 1) * NI)
        dens[sl] = s * xfac[sl]
    return dens.reshape(N, 1).astype(np.float32)


if __name__ == "__main__":
    rng = np.random.default_rng(0)
    x = rng.standard_normal((N, D), dtype=np.float32)
    data = rng.standard_normal((M, D), dtype=np.float32)
    out = kernel(x, data)
    print("kernel out", out.shape, out[:4, 0])
